# revision 1
# baseline (speedup 1.0000x reference)
"""Multi-head attention (4x2048x1024, 16 heads) on 8 TRN2 NeuronCores.

Sharding: core c handles batch c//2, query seq-half c%2 (1024 queries).
Each core computes QKV projection for its own seq half plus K/V for the
peer half (redundant compute instead of a 2-rank collective), full
attention for all 16 heads over its 1024 queries x 2048 keys, and the
output projection. Outputs are disjoint -> no collectives; host concats.

Host passes transposed (d-major) shards with the core's own seq-half
first, so the SPMD graph is identical on every core.
"""

import numpy as np

import concourse.mybir as mybir
import concourse.tile as tile
from concourse import bacc
from concourse.bass_utils import run_bass_kernel_spmd
FP32 = mybir.dt.float32
BF16 = mybir.dt.bfloat16

DIM = 1024
HEADS = 16
HD = 64
AUG = HD + 1  # V columns per head + ones column for sum-exp
SCALE = DIM ** -0.5
SEQ = 2048
NI = 1024  # queries per core
NJ = 2048  # keys per core
B = 4
N_CORES = 8
P = 128

TRACE = False
LAST_RESULTS = None
_NC_CACHE = None


def _build():
    nc = bacc.Bacc(
        "TRN2",
        target_bir_lowering=False,
        debug=False,
        enable_asserts=False,
        num_devices=N_CORES,
    )
    xT = nc.dram_tensor("xT", [DIM, NJ], FP32, kind="ExternalInput")
    wqkvT = nc.dram_tensor("wqkvT", [DIM, 3 * DIM], FP32, kind="ExternalInput")
    woutT = nc.dram_tensor("woutT", [DIM, DIM], FP32, kind="ExternalInput")
    bout = nc.dram_tensor("bout", [1, DIM], FP32, kind="ExternalInput")
    out = nc.dram_tensor("out", [NI, DIM], FP32, kind="ExternalOutput")

    ND = DIM // P  # 8 contraction tiles

    with tile.TileContext(nc) as tc:
        with (
            tc.tile_pool(name="persist", bufs=1) as persist,
            tc.tile_pool(name="stage", bufs=3) as stage,
            tc.tile_pool(name="wpool", bufs=9) as wpool,
            tc.tile_pool(name="sb", bufs=3) as sb,
            tc.tile_pool(name="small", bufs=3) as small,
            tc.tile_pool(name="ps", bufs=4, space="PSUM") as psp,
        ):
            xpool_cm = tc.tile_pool(name="xpool", bufs=1)
            xpool = xpool_cm.__enter__()

            # ---- bias broadcast [1,1024] -> [128,1024]
            bias_sb = small.tile([1, DIM], FP32, tag="bias", name="bias", bufs=1)
            nc.sync.dma_start(out=bias_sb, in_=bout.ap())
            bias_bc = small.tile([P, DIM], FP32, tag="biasbc", name="biasbc", bufs=1)
            nc.gpsimd.partition_broadcast(bias_bc, bias_sb)

            def load_w_group(src_ap, ebase):
                """Load+cast 8 weight tiles [128 d, 1024 e] for one group."""
                grp = []
                for dt in range(ND):
                    ws = stage.tile([P, DIM], FP32, tag="stg", name="stg")
                    nc.sync.dma_start(
                        out=ws,
                        in_=src_ap[dt * P:(dt + 1) * P, ebase:ebase + DIM],
                    )
                    wb = wpool.tile([P, DIM], BF16, tag="wbf", name="wbf")
                    nc.vector.tensor_copy(wb, ws)
                    grp.append(wb)
                return grp

            def load_x_half(xbf, half):
                for dt in range(ND):
                    xs = stage.tile([P, DIM], FP32, tag="stg", name="stg")
                    nc.sync.dma_start(
                        out=xs,
                        in_=xT.ap()[dt * P:(dt + 1) * P,
                                    half * DIM:(half + 1) * DIM],
                    )
                    dst = xbf[dt][:, half * DIM:(half + 1) * DIM]
                    if (dt + half) % 2 == 0:
                        nc.vector.tensor_copy(dst, xs)
                    else:
                        nc.scalar.copy(dst, xs)

            # ---- Q first (own-half x + Q weights only) so PE starts early
            xbf = [xpool.tile([P, NJ], BF16, tag=f"xbf{dt}", name=f"xbf{dt}")
                   for dt in range(ND)]
            wq = []
            for dt in range(ND):
                ws = stage.tile([P, DIM], FP32, tag="stg", name="stg")
                nc.sync.dma_start(
                    out=ws, in_=wqkvT.ap()[dt * P:(dt + 1) * P, 0:DIM])
                wb = wpool.tile([P, DIM], BF16, tag="wbf", name="wbf")
                nc.vector.tensor_copy(wb, ws)
                wq.append(wb)
                xs = stage.tile([P, DIM], FP32, tag="stg", name="stg")
                nc.sync.dma_start(
                    out=xs, in_=xT.ap()[dt * P:(dt + 1) * P, 0:DIM])
                nc.scalar.copy(xbf[dt][:, 0:DIM], xs)

            qt = [persist.tile([P, NI], BF16, tag=f"qt{e}", name=f"qt{e}")
                  for e in range(8)]
            kt = [persist.tile([P, NJ], BF16, tag=f"kt{e}", name=f"kt{e}")
                  for e in range(8)]

            def qk_proj(wg, tiles, chunks):
                # e-major out [e, n]; stationary (weight) reused per chunk set
                for et in range(8):
                    pss = {ch: psp.tile([P, DIM], FP32, tag="ps", name="ps")
                           for ch in chunks}
                    for dt in range(ND):
                        for ch in chunks:
                            for sc in range(2):
                                nb = ch * DIM + sc * 512
                                nc.tensor.matmul(
                                    pss[ch][:, sc * 512:(sc + 1) * 512],
                                    wg[dt][:, et * P:(et + 1) * P],
                                    xbf[dt][:, nb:nb + 512],
                                    start=(dt == 0),
                                    stop=(dt == ND - 1),
                                )
                    for ch in chunks:
                        dst = tiles[et][:, ch * DIM:(ch + 1) * DIM]
                        if (et + ch) % 2 == 0:
                            nc.vector.tensor_copy(dst, pss[ch])
                        else:
                            nc.scalar.copy(dst, pss[ch])

            qk_proj(wq, qt, [0])

            load_x_half(xbf, 1)
            wk = load_w_group(wqkvT.ap(), DIM)
            qk_proj(wk, kt, [0, 1])

            # ---- V projection: out n-major [n, e], scattered into 65-wide
            # per-head blocks with a ones column at offset 64 (sum-exp trick)
            vaug = [persist.tile([P, HEADS * AUG], BF16, tag=f"va{j}", name=f"va{j}")
                    for j in range(16)]
            for jt in range(16):
                v3 = vaug[jt].rearrange("p (h c) -> p h c", c=AUG)
                nc.vector.memset(v3[:, :, HD:AUG], 1.0)
            wv = load_w_group(wqkvT.ap(), 2 * DIM)

            def v_proj(jts):
                for jt in jts:
                    ps = psp.tile([P, DIM], FP32, tag="ps", name="ps")
                    for dt in range(ND):
                        for sc in range(2):  # e-chunks of 512 = 8 heads each
                            nc.tensor.matmul(
                                ps[:, sc * 512:(sc + 1) * 512],
                                xbf[dt][:, jt * P:(jt + 1) * P],
                                wv[dt][:, sc * 512:(sc + 1) * 512],
                                start=(dt == 0),
                                stop=(dt == ND - 1),
                            )
                    vsrc = ps.rearrange("p (h c) -> p h c", c=HD)
                    vdst = vaug[jt].rearrange("p (h c) -> p h c", c=AUG)[:, :, 0:HD]
                    nc.scalar.copy(vdst, vsrc)

            v_proj(range(16))

            # prefetch out-projection weights; DMAs+casts run during attention
            wo = load_w_group(woutT.ap(), 0)

            xpool_cm.__exit__(None, None, None)
            norm_cm = tc.tile_pool(name="norm", bufs=2)
            norm = norm_cm.__enter__()

            # ---- attention, head pairs (2hp, 2hp+1) share e-tile hp.
            # Pair 0 is emitted before the peer-half K/V projections: its
            # jt 0-7 (own half) can start as soon as own-half proj is done,
            # so the ScalarE exp stream starts ~60us earlier; the peer
            # projections then overlap pair 0's jt 8-15 dependencies.
            aot = [persist.tile([P, NI], BF16, tag=f"ao{e}", name=f"ao{e}")
                   for e in range(8)]

            def attention_pair(hp, jts=range(16), avs=None):
                if avs is None:
                    avA = psp.tile([AUG, NI], FP32, tag="ps", name="av")
                    avB = psp.tile([AUG, NI], FP32, tag="ps", name="av")
                else:
                    avA, avB = avs
                for jt in jts:
                    dA = psp.tile([P, NI], FP32, tag="ps", name="dots")
                    dB = psp.tile([P, NI], FP32, tag="ps", name="dots")
                    jsl = slice(jt * P, (jt + 1) * P)
                    # alternate row groups (A at rows 0-63, B at 64-127) so
                    # consecutive dots run concurrently on disjoint sub-arrays
                    for ic in range(2):
                        isl = slice(ic * 512, (ic + 1) * 512)
                        nc.tensor.matmul(
                            dA[:, isl], kt[hp][0:HD, jsl], qt[hp][0:HD, isl],
                            start=True, stop=True,
                        )
                        nc.tensor.matmul(
                            dB[:, isl], kt[hp][HD:P, jsl], qt[hp][HD:P, isl],
                            start=True, stop=True,
                        )
                    eA = sb.tile([P, NI], BF16, tag="expT", name="expT", bufs=3)
                    nc.scalar.activation(eA, dA, mybir.ActivationFunctionType.Exp,
                                         scale=SCALE)
                    eB = sb.tile([P, NI], BF16, tag="expT", name="expT", bufs=3)
                    nc.scalar.activation(eB, dB, mybir.ActivationFunctionType.Exp,
                                         scale=SCALE)
                    first, last = jt == 0, jt == 15
                    for av, e_t, head in ((avA, eA, 2 * hp), (avB, eB, 2 * hp + 1)):
                        for ic in range(2):
                            isl = slice(ic * 512, (ic + 1) * 512)
                            nc.tensor.matmul(
                                av[:, isl],
                                vaug[jt][:, head * AUG:(head + 1) * AUG],
                                e_t[:, isl],
                                start=first, stop=last,
                            )
                    # PE "heater" burst: a short run of full 128x128 matmuls
                    # with no concurrent PSUM readers restores the PE from the
                    # degraded state that sustained ACT-PSUM-read contention
                    # latches it into (measured: bursts net ~25% faster
                    # attention despite the wasted work).
                    if jt in (7, 15):
                        for _ in range(4):
                            pb = psp.tile([P, NI], FP32, tag="ps", name="heat")
                            for sc in range(2):
                                nc.tensor.matmul(
                                    pb[:, sc * 512:(sc + 1) * 512],
                                    kt[hp][:, 0:P],
                                    qt[hp][:, 0:512],
                                    start=True, stop=True,
                                )
                    # PE "heater" burst: a short run of full 128x128 matmuls
                    # with no concurrent PSUM readers restores the PE from the
                    # degraded state that sustained ACT-PSUM-read contention
                    # latches it into (measured: bursts net ~25% faster
                    # attention despite the wasted work).
                    if jt in (7, 15):
                        for _ in range(4):
                            pb = psp.tile([P, NI], FP32, tag="ps", name="heat")
                            for sc in range(2):
                                nc.tensor.matmul(
                                    pb[:, sc * 512:(sc + 1) * 512],
                                    kt[hp][:, 0:P],
                                    qt[hp][:, 0:512],
                                    start=True, stop=True,
                                )
                if 15 not in jts:
                    return (avA, avB)
                # per-pair softmax normalization, overlapped with the next
                # pair's attention. Sum-exp rows gathered at partitions 0/32
                # (32-aligned as DVE requires), one reciprocal per pair;
                # partition_broadcast only accepts base-0 inputs, so head B's
                # row goes through a base-0 temp.
                g = norm.tile([33, NI], FP32, tag="g", name="g")
                nc.vector.tensor_copy(aot[hp][0:HD, :], avA[0:HD, :])
                nc.vector.tensor_copy(aot[hp][HD:P, :], avB[0:HD, :])
                nc.vector.tensor_copy(g[0:1, :], avA[HD:AUG, :])
                nc.vector.tensor_copy(g[32:33, :], avB[HD:AUG, :])
                rp = norm.tile([33, NI], FP32, tag="rp", name="rp")
                nc.vector.reciprocal(rp, g)  # rows 1..31 junk, unused
                rbA = norm.tile([P, NI], FP32, tag="rb", name="rb")
                nc.gpsimd.partition_broadcast(rbA, rp[0:1, :])
                nc.gpsimd.tensor_mul(
                    aot[hp][0:HD, :], aot[hp][0:HD, :], rbA[0:HD, :])
                tb = norm.tile([1, NI], FP32, tag="tb", name="tb")
                nc.vector.tensor_copy(tb, rp[32:33, :])
                rbB = norm.tile([P, NI], FP32, tag="rb", name="rb")
                nc.gpsimd.partition_broadcast(rbB, tb)
                nc.gpsimd.tensor_mul(
                    aot[hp][HD:P, :], aot[hp][HD:P, :], rbB[HD:P, :])

            for hp in range(8):
                attention_pair(hp)
            # prefetch out-projection weights; DMAs+casts run during attention
            wo = load_w_group(woutT.ap(), 0)

            norm_cm.__exit__(None, None, None)

            # ---- output projection + bias
            for it in range(8):
                ps = psp.tile([P, DIM], FP32, tag="ps", name="ps")
                for et in range(8):
                    for fc in range(2):
                        fsl = slice(fc * 512, (fc + 1) * 512)
                        nc.tensor.matmul(
                            ps[:, fsl],
                            aot[et][:, it * P:(it + 1) * P],
                            wo[et][:, fsl],
                            start=(et == 0),
                            stop=(et == 7),
                        )
                osb = sb.tile([P, DIM], FP32, tag="outsb", name="outsb", bufs=2)
                nc.vector.tensor_add(osb, ps, bias_bc)
                nc.sync.dma_start(out=out.ap()[it * P:(it + 1) * P, :], in_=osb)

    nc.compile()
    return nc


def _get_nc():
    global _NC_CACHE
    if _NC_CACHE is None:
        _NC_CACHE = _build()
    return _NC_CACHE


def kernel(x, w_qkv, w_out, b_out):
    global LAST_RESULTS
    x = np.asarray(x, dtype=np.float32)
    w_qkv = np.asarray(w_qkv, dtype=np.float32)
    w_out = np.asarray(w_out, dtype=np.float32)
    b_out = np.asarray(b_out, dtype=np.float32)

    nc = _get_nc()

    wqkvT = np.ascontiguousarray(w_qkv.T)
    woutT = np.ascontiguousarray(w_out.T)
    brow = np.ascontiguousarray(b_out.reshape(1, DIM))

    in_maps = []
    for c in range(N_CORES):
        b, h = divmod(c, 2)
        own = x[b, h * NI:(h + 1) * NI, :]
        peer = x[b, (1 - h) * NI:(2 - h) * NI, :]
        xTc = np.ascontiguousarray(np.concatenate([own, peer], axis=0).T)
        in_maps.append({
            "xT": xTc,
            "wqkvT": wqkvT,
            "woutT": woutT,
            "bout": brow,
        })

    res = run_bass_kernel_spmd(
        nc, in_maps, core_ids=list(range(N_CORES)), trace=TRACE
    )
    LAST_RESULTS = res

    out = np.empty((B, SEQ, DIM), dtype=np.float32)
    for c in range(N_CORES):
        b, h = divmod(c, 2)
        out[b, h * NI:(h + 1) * NI, :] = res.results[c]["out"]
    return out



# revision 12
# speedup vs baseline: 1.0751x; 1.0751x over previous
"""Multi-head attention (4x2048x1024, 16 heads) on 8 TRN2 NeuronCores.

Sharding: core c handles batch c//2, query seq-half c%2 (1024 queries).
Each core computes QKV projection for its own seq half plus K/V for the
peer half (redundant compute instead of a 2-rank collective), full
attention for all 16 heads over its 1024 queries x 2048 keys, and the
output projection. Outputs are disjoint -> no collectives; host concats.

v2 vs baseline:
- host pre-casts inputs to bf16: half the DMA bytes, no on-device
  stage+cast pipeline (frees DVE, removes weight-load stalls)
- heater bursts removed; ACT does exps only (proj copies on DVE/Pool)
- projection phase keeps PE saturated; pair-0 dots+exps interleaved
  into it with exp tiles buffered in SBUF (jt 0-7), so ACT starts early
- attention phase: per-pair lookahead-dots emission with dots PSUM
  double-buffered (pd 2x2 banks) + av pair (pav 2x2 banks) -> ACT
  (the bottleneck there) never waits on PSUM rotation
"""

import numpy as np

import concourse.mybir as mybir
import concourse.tile as tile
from concourse import bacc
from concourse.bass_utils import run_bass_kernel_spmd
FP32 = mybir.dt.float32
BF16 = mybir.dt.bfloat16

DIM = 1024
HEADS = 16
HD = 64
AUG = HD + 1  # V columns per head + ones column for sum-exp
SCALE = DIM ** -0.5
SEQ = 2048
NI = 1024  # queries per core
NJ = 2048  # keys per core
B = 4
N_CORES = 8
P = 128
ND = DIM // P  # 8 contraction tiles
NBUF = 8  # pair-0 jts with SBUF-buffered exps

TRACE = False
LAST_RESULTS = None
_NC_CACHE = None


def _build():
    nc = bacc.Bacc(
        "TRN2",
        target_bir_lowering=False,
        debug=False,
        enable_asserts=False,
        num_devices=N_CORES,
    )
    # all inputs pre-cast/transposed by host
    xT = nc.dram_tensor("xT", [DIM, NJ], BF16, kind="ExternalInput")
    wqkvT = nc.dram_tensor("wqkvT", [DIM, 3 * DIM], BF16, kind="ExternalInput")
    woutT = nc.dram_tensor("woutT", [DIM, DIM], BF16, kind="ExternalInput")
    bout = nc.dram_tensor("bout", [1, DIM], FP32, kind="ExternalInput")
    out = nc.dram_tensor("out", [NI, DIM], FP32, kind="ExternalOutput")

    with tile.TileContext(nc) as tc:
        with (
            tc.tile_pool(name="persist", bufs=1) as persist,
            tc.tile_pool(name="sb", bufs=3) as sb,
            tc.tile_pool(name="small", bufs=3) as small,
        ):
            e0pool_cm = tc.tile_pool(name="e0pool", bufs=1)
            e0pool = e0pool_cm.__enter__()
            xpool_cm = tc.tile_pool(name="xpool", bufs=1)
            xpool = xpool_cm.__enter__()
            wpool_cm = tc.tile_pool(name="wpool", bufs=1)
            wpool = wpool_cm.__enter__()
            pp_cm = tc.tile_pool(name="pp", bufs=4, space="PSUM")
            pp = pp_cm.__enter__()

            # ---- bias broadcast [1,1024] -> [128,1024]
            bias_sb = small.tile([1, DIM], FP32, tag="bias", name="bias", bufs=1)
            nc.sync.dma_start(out=bias_sb, in_=bout.ap())
            bias_bc = small.tile([P, DIM], FP32, tag="biasbc", name="biasbc", bufs=1)
            nc.gpsimd.partition_broadcast(bias_bc, bias_sb)

            # ---- persistent tiles
            xbf = [xpool.tile([P, NJ], BF16, tag=f"xbf{dt}", name=f"xbf{dt}")
                   for dt in range(ND)]
            qt = [persist.tile([P, NI], BF16, tag=f"qt{e}", name=f"qt{e}")
                  for e in range(8)]
            kt = [persist.tile([P, NJ], BF16, tag=f"kt{e}", name=f"kt{e}")
                  for e in range(8)]
            vaug = [persist.tile([P, HEADS * AUG], BF16, tag=f"va{j}",
                                 name=f"va{j}") for j in range(16)]

            # weights: wv full-width (v_proj uses all heads per jt); wq/wk
            # sliced per e-tile [128, (dt 8) x 128] (2KB/part) in rotating
            # pools, one strided DMA per e-tile, loaded just-in-time.
            wv = [wpool.tile([P, DIM], BF16, tag=f"wv{dt}", name=f"wv{dt}")
                  for dt in range(ND)]

            def load_we(ebase, et):
                """[128 p, 8 dt, 128 cols]: [p, dt, c] = wqkvT[dt*128+p,
                ebase + et*128 + c] -- one DMA for all 8 dt sub-tiles."""
                w = wpool.tile([P, ND, P], BF16, tag="we", name="we", bufs=5)
                nc.sync.dma_start(
                    out=w,
                    in_=wqkvT.ap()
                    .rearrange("(dt p) e -> p dt e", p=P)
                    [:, :, ebase + et * P: ebase + (et + 1) * P],
                )
                return w

            # DMA order = priority: x own half first (Q proj starts the PE),
            # then x peer, then wv.
            for dt in range(ND):
                nc.sync.dma_start(
                    out=xbf[dt][:, 0:DIM],
                    in_=xT.ap()[dt * P:(dt + 1) * P, 0:DIM])
            for dt in range(ND):
                nc.sync.dma_start(
                    out=xbf[dt][:, DIM:NJ],
                    in_=xT.ap()[dt * P:(dt + 1) * P, DIM:NJ])
            for dt in range(ND):
                nc.sync.dma_start(
                    out=wv[dt],
                    in_=wqkvT.ap()[dt * P:(dt + 1) * P, 2 * DIM:3 * DIM])

            # ones columns of vaug (sum-exp trick)
            for jt in range(16):
                v3 = vaug[jt].rearrange("p (h c) -> p h c", c=AUG)
                nc.vector.memset(v3[:, :, HD:AUG], 1.0)

            def qk_proj(we, tiles, et, chunks):
                """Project one e-tile (2 heads) for the given seq chunks."""
                pss = {ch: pp.tile([P, DIM], FP32, tag="pp", name="pp")
                       for ch in chunks}
                for dt in range(ND):
                    for ch in chunks:
                        for sc in range(2):
                            nb = ch * DIM + sc * 512
                            nc.tensor.matmul(
                                pss[ch][:, sc * 512:(sc + 1) * 512],
                                we[:, dt, :],
                                xbf[dt][:, nb:nb + 512],
                                start=(dt == 0),
                                stop=(dt == ND - 1),
                            )
                for ch in chunks:
                    dst = tiles[et][:, ch * DIM:(ch + 1) * DIM]
                    if (et + ch) % 2 == 0:
                        nc.vector.tensor_copy(dst, pss[ch])
                    else:
                        nc.scalar.copy(dst, pss[ch])

            def v_proj(jt):
                ps = pp.tile([P, DIM], FP32, tag="pp", name="pp")
                for dt in range(ND):
                    for sc in range(2):  # e-chunks of 512 = 8 heads each
                        nc.tensor.matmul(
                            ps[:, sc * 512:(sc + 1) * 512],
                            xbf[dt][:, jt * P:(jt + 1) * P],
                            wv[dt][:, sc * 512:(sc + 1) * 512],
                            start=(dt == 0),
                            stop=(dt == ND - 1),
                        )
                vsrc = ps.rearrange("p (h c) -> p h c", c=HD)
                vdst = vaug[jt].rearrange("p (h c) -> p h c", c=AUG)[:, :, 0:HD]
                if jt % 2 == 0:
                    nc.vector.tensor_copy(vdst, vsrc)
                else:
                    nc.scalar.copy(vdst, vsrc)

            def emit_dots(pool, tag, hp, jt, half):
                """One head's dots [128 keys, 1024 queries] for key-tile jt.
                half 0 -> PE rows 0:64, half 1 -> rows 64:128; consecutive
                halves run concurrently on disjoint PE row ranges."""
                d = pool.tile([P, NI], FP32, tag=tag, name="dots")
                jsl = slice(jt * P, (jt + 1) * P)
                rsl = slice(0, HD) if half == 0 else slice(HD, P)
                for ic in range(2):
                    isl = slice(ic * 512, (ic + 1) * 512)
                    nc.tensor.matmul(
                        d[:, isl], kt[hp][rsl, jsl], qt[hp][rsl, isl],
                        start=True, stop=True,
                    )
                return d

            def emit_exp(d, e_t):
                nc.scalar.activation(e_t, d, mybir.ActivationFunctionType.Exp,
                                     scale=SCALE)

            # ---- projection phase, with pair-0 dots+exp (jt 0..NBUF-1)
            # interleaved; exps buffered in SBUF for AV replay later.
            wq0 = load_we(0, 0)
            wk0 = load_we(DIM, 0)
            qk_proj(wq0, qt, 0, [0])
            qk_proj(wk0, kt, 0, [0, 1])

            e0 = [e0pool.tile([P, NI], BF16, tag=f"e0_{j}", name=f"e0_{j}")
                  for j in range(2 * NBUF)]
            wjobs = [(0, e) for e in range(1, 8)] + \
                    [(DIM, e) for e in range(1, 8)]  # 14: Q et1-7, K et1-7
            wnext = load_we(*wjobs[0])
            for jt in range(16):
                if jt < NBUF:
                    dA = emit_dots(pp, 'pp', 0, jt, 0)
                    dB = emit_dots(pp, 'pp', 0, jt, 1)
                    emit_exp(dA, e0[2 * jt])
                    emit_exp(dB, e0[2 * jt + 1])
                v_proj(jt)
                if jt < 14:
                    wcur = wnext
                    if jt + 1 < 14:
                        wnext = load_we(*wjobs[jt + 1])
                    if jt < 7:
                        qk_proj(wcur, qt, jt + 1, [0])
                    else:
                        qk_proj(wcur, kt, jt - 6, [0, 1])

            pp_cm.__exit__(None, None, None)
            wpool_cm.__exit__(None, None, None)
            xpool_cm.__exit__(None, None, None)

            # ---- attention-phase pools (av pair 4 banks + dots 4 banks)
            late_cm = tc.tile_pool(name="late", bufs=1)
            late = late_cm.__enter__()
            aot = [late.tile([P, NI], BF16, tag=f"ao{e}", name=f"ao{e}")
                   for e in range(8)]
            wo = [late.tile([P, DIM], BF16, tag=f"wo{dt}", name=f"wo{dt}")
                  for dt in range(ND)]
            for dt in range(ND):
                nc.sync.dma_start(
                    out=wo[dt], in_=woutT.ap()[dt * P:(dt + 1) * P, :])

            norm_cm = tc.tile_pool(name="norm", bufs=2)
            norm = norm_cm.__enter__()
            pd_cm = tc.tile_pool(name="pd", bufs=2, space="PSUM")
            pd = pd_cm.__enter__()
            pav_cm = tc.tile_pool(name="pav", bufs=2, space="PSUM")
            pav = pav_cm.__enter__()

            def av_accum(av, e_t, jt, head):
                first, last = jt == 0, jt == 15
                for ic in range(2):
                    isl = slice(ic * 512, (ic + 1) * 512)
                    nc.tensor.matmul(
                        av[:, isl],
                        vaug[jt][:, head * AUG:(head + 1) * AUG],
                        e_t[:, isl],
                        start=first, stop=last,
                    )

            def normalize(hp, avA, avB):
                # per-pair softmax normalization (overlaps next pair).
                # Sum-exp rows gathered at partitions 0/32 (32-aligned as DVE
                # requires), one reciprocal per pair; partition_broadcast only
                # accepts base-0 inputs, so head B's row uses a base-0 temp.
                g = norm.tile([33, NI], FP32, tag="g", name="g")
                nc.vector.tensor_copy(aot[hp][0:HD, :], avA[0:HD, :])
                nc.vector.tensor_copy(aot[hp][HD:P, :], avB[0:HD, :])
                nc.vector.tensor_copy(g[0:1, :], avA[HD:AUG, :])
                nc.vector.tensor_copy(g[32:33, :], avB[HD:AUG, :])
                rp = norm.tile([33, NI], FP32, tag="rp", name="rp")
                nc.vector.reciprocal(rp, g)  # rows 1..31 junk, unused
                rbA = norm.tile([P, NI], FP32, tag="rb", name="rb")
                nc.gpsimd.partition_broadcast(rbA, rp[0:1, :])
                nc.gpsimd.tensor_mul(
                    aot[hp][0:HD, :], aot[hp][0:HD, :], rbA[0:HD, :])
                tb = norm.tile([1, NI], FP32, tag="tb", name="tb")
                nc.vector.tensor_copy(tb, rp[32:33, :])
                rbB = norm.tile([P, NI], FP32, tag="rb", name="rb")
                nc.gpsimd.partition_broadcast(rbB, tb)
                nc.gpsimd.tensor_mul(
                    aot[hp][HD:P, :], aot[hp][HD:P, :], rbB[HD:P, :])

            # ---- attention phase.
            # pair 0: AV replay of buffered jts + streamed jts NBUF..15.
            avA = pav.tile([AUG, NI], FP32, tag="pav", name="av")
            avB = pav.tile([AUG, NI], FP32, tag="pav", name="av")
            dA = emit_dots(pd, 'pd', 0, NBUF, 0)
            dB = emit_dots(pd, 'pd', 0, NBUF, 1)
            # replay first so AV start=True lands on jt 0 in PE order
            av_accum(avA, e0[0], 0, 0)
            av_accum(avB, e0[1], 0, 1)
            for jt in range(NBUF, 16):
                eA = sb.tile([P, NI], BF16, tag="expT", name="expT", bufs=4)
                eB = sb.tile([P, NI], BF16, tag="expT", name="expT", bufs=4)
                emit_exp(dA, eA)
                emit_exp(dB, eB)
                r = jt - NBUF + 1  # replay index
                if jt < 15:
                    dA = emit_dots(pd, 'pd', 0, jt + 1, 0)
                if r < NBUF:
                    av_accum(avA, e0[2 * r], r, 0)
                    av_accum(avB, e0[2 * r + 1], r, 1)
                av_accum(avA, eA, jt, 0)
                if jt < 15:
                    dB = emit_dots(pd, 'pd', 0, jt + 1, 1)
                av_accum(avB, eB, jt, 1)
            normalize(0, avA, avB)

            # pairs 1-7: lookahead-dots pipeline
            for hp in range(1, 8):
                avA = pav.tile([AUG, NI], FP32, tag="pav", name="av")
                avB = pav.tile([AUG, NI], FP32, tag="pav", name="av")
                dA = emit_dots(pd, 'pd', hp, 0, 0)
                dB = emit_dots(pd, 'pd', hp, 0, 1)
                for jt in range(16):
                    eA = sb.tile([P, NI], BF16, tag="expT", name="expT", bufs=4)
                    eB = sb.tile([P, NI], BF16, tag="expT", name="expT", bufs=4)
                    emit_exp(dA, eA)
                    emit_exp(dB, eB)
                    if jt < 15:
                        dA = emit_dots(pd, 'pd', hp, jt + 1, 0)
                    av_accum(avA, eA, jt, 2 * hp)
                    if jt < 15:
                        dB = emit_dots(pd, 'pd', hp, jt + 1, 1)
                    av_accum(avB, eB, jt, 2 * hp + 1)
                normalize(hp, avA, avB)

            pav_cm.__exit__(None, None, None)
            pd_cm.__exit__(None, None, None)
            norm_cm.__exit__(None, None, None)

            # ---- output projection + bias
            po_cm = tc.tile_pool(name="po", bufs=3, space="PSUM")
            po = po_cm.__enter__()
            for it in range(8):
                ps = po.tile([P, DIM], FP32, tag="po", name="po")
                for et in range(8):
                    for fc in range(2):
                        fsl = slice(fc * 512, (fc + 1) * 512)
                        nc.tensor.matmul(
                            ps[:, fsl],
                            aot[et][:, it * P:(it + 1) * P],
                            wo[et][:, fsl],
                            start=(et == 0),
                            stop=(et == 7),
                        )
                osb = sb.tile([P, DIM], FP32, tag="outsb", name="outsb", bufs=2)
                nc.vector.tensor_add(osb, ps, bias_bc)
                nc.sync.dma_start(out=out.ap()[it * P:(it + 1) * P, :], in_=osb)
            po_cm.__exit__(None, None, None)
            late_cm.__exit__(None, None, None)
            e0pool_cm.__exit__(None, None, None)

    nc.compile()
    return nc


def _get_nc():
    global _NC_CACHE
    if _NC_CACHE is None:
        _NC_CACHE = _build()
    return _NC_CACHE


def kernel(x, w_qkv, w_out, b_out):
    global LAST_RESULTS
    import ml_dtypes
    BF = ml_dtypes.bfloat16
    x = np.asarray(x, dtype=np.float32)
    w_qkv = np.asarray(w_qkv, dtype=np.float32)
    w_out = np.asarray(w_out, dtype=np.float32)
    b_out = np.asarray(b_out, dtype=np.float32)

    nc = _get_nc()

    wqkvT = np.ascontiguousarray(w_qkv.T.astype(BF))
    woutT = np.ascontiguousarray(w_out.T.astype(BF))
    brow = np.ascontiguousarray(b_out.reshape(1, DIM))

    in_maps = []
    for c in range(N_CORES):
        b, h = divmod(c, 2)
        own = x[b, h * NI:(h + 1) * NI, :]
        peer = x[b, (1 - h) * NI:(2 - h) * NI, :]
        xTc = np.ascontiguousarray(
            np.concatenate([own, peer], axis=0).T.astype(BF))
        in_maps.append({
            "xT": xTc,
            "wqkvT": wqkvT,
            "woutT": woutT,
            "bout": brow,
        })

    res = run_bass_kernel_spmd(
        nc, in_maps, core_ids=list(range(N_CORES)), trace=TRACE
    )
    LAST_RESULTS = res

    out = np.empty((B, SEQ, DIM), dtype=np.float32)
    for c in range(N_CORES):
        b, h = divmod(c, 2)
        out[b, h * NI:(h + 1) * NI, :] = res.results[c]["out"]
    return out


# revision 13
# speedup vs baseline: 1.1694x; 1.0877x over previous
"""Multi-head attention (4x2048x1024, 16 heads) on 8 TRN2 NeuronCores.

Sharding: core c handles batch c//2, query seq-half c%2 (1024 queries).
Each core computes QKV projection for its own seq half plus K/V for the
peer half (redundant compute instead of a 2-rank collective), full
attention for all 16 heads over its 1024 queries x 2048 keys, and the
output projection. Outputs are disjoint -> no collectives; host concats.

v2 vs baseline:
- host pre-casts inputs to bf16: half the DMA bytes, no on-device
  stage+cast pipeline (frees DVE, removes weight-load stalls)
- heater bursts removed; ACT does exps only (proj copies on DVE/Pool)
- projection phase keeps PE saturated; pair-0 dots+exps interleaved
  into it with exp tiles buffered in SBUF (jt 0-7), so ACT starts early
- attention phase: per-pair lookahead-dots emission with dots PSUM
  double-buffered (pd 2x2 banks) + av pair (pav 2x2 banks) -> ACT
  (the bottleneck there) never waits on PSUM rotation
"""

import numpy as np

import concourse.mybir as mybir
import concourse.tile as tile
from concourse import bacc
from concourse.bass_utils import run_bass_kernel_spmd
FP32 = mybir.dt.float32
BF16 = mybir.dt.bfloat16

DIM = 1024
HEADS = 16
HD = 64
AUG = HD + 1  # V columns per head + ones column for sum-exp
SCALE = DIM ** -0.5
SEQ = 2048
NI = 1024  # queries per core
NJ = 2048  # keys per core
B = 4
N_CORES = 8
P = 128
ND = DIM // P  # 8 contraction tiles
NBUF = 8  # pair-0 jts with SBUF-buffered exps

TRACE = False
LAST_RESULTS = None
_NC_CACHE = None


def _build():
    nc = bacc.Bacc(
        "TRN2",
        target_bir_lowering=False,
        debug=False,
        enable_asserts=False,
        num_devices=N_CORES,
    )
    # all inputs pre-cast/transposed by host
    xT = nc.dram_tensor("xT", [DIM, NJ], BF16, kind="ExternalInput")
    wqkvT = nc.dram_tensor("wqkvT", [DIM, 3 * DIM], BF16, kind="ExternalInput")
    woutT = nc.dram_tensor("woutT", [DIM, DIM], BF16, kind="ExternalInput")
    bout = nc.dram_tensor("bout", [1, DIM], FP32, kind="ExternalInput")
    out = nc.dram_tensor("out", [NI, DIM], FP32, kind="ExternalOutput")

    with tile.TileContext(nc) as tc:
        with (
            tc.tile_pool(name="persist", bufs=1) as persist,
            tc.tile_pool(name="sb", bufs=3) as sb,
            tc.tile_pool(name="small", bufs=3) as small,
        ):
            e0pool_cm = tc.tile_pool(name="e0pool", bufs=1)
            e0pool = e0pool_cm.__enter__()
            xpool_cm = tc.tile_pool(name="xpool", bufs=1)
            xpool = xpool_cm.__enter__()
            wpool_cm = tc.tile_pool(name="wpool", bufs=1)
            wpool = wpool_cm.__enter__()
            pp_cm = tc.tile_pool(name="pp", bufs=4, space="PSUM")
            pp = pp_cm.__enter__()

            # ---- bias broadcast [1,1024] -> [128,1024]
            bias_sb = small.tile([1, DIM], FP32, tag="bias", name="bias", bufs=1)
            nc.sync.dma_start(out=bias_sb, in_=bout.ap())
            bias_bc = small.tile([P, DIM], FP32, tag="biasbc", name="biasbc", bufs=1)
            nc.gpsimd.partition_broadcast(bias_bc, bias_sb)

            # ---- persistent tiles
            xbf = [xpool.tile([P, NJ], BF16, tag=f"xbf{dt}", name=f"xbf{dt}")
                   for dt in range(ND)]
            qt = [persist.tile([P, NI], BF16, tag=f"qt{e}", name=f"qt{e}")
                  for e in range(8)]
            kt = [persist.tile([P, NJ], BF16, tag=f"kt{e}", name=f"kt{e}")
                  for e in range(8)]
            vaug = [persist.tile([P, HEADS * AUG], BF16, tag=f"va{j}",
                                 name=f"va{j}") for j in range(16)]

            # weights: wv full-width (v_proj uses all heads per jt); wq/wk
            # sliced per e-tile [128, (dt 8) x 128] (2KB/part) in rotating
            # pools, one strided DMA per e-tile, loaded just-in-time.
            wv = [wpool.tile([P, DIM], BF16, tag=f"wv{dt}", name=f"wv{dt}")
                  for dt in range(ND)]

            def load_we(ebase, et):
                """[128 p, 8 dt, 128 cols]: [p, dt, c] = wqkvT[dt*128+p,
                ebase + et*128 + c] -- one DMA for all 8 dt sub-tiles."""
                w = wpool.tile([P, ND, P], BF16, tag="we", name="we", bufs=5)
                nc.sync.dma_start(
                    out=w,
                    in_=wqkvT.ap()
                    .rearrange("(dt p) e -> p dt e", p=P)
                    [:, :, ebase + et * P: ebase + (et + 1) * P],
                )
                return w

            # DMA order = priority: x own half first (Q proj starts the PE);
            # x peer + wv are emitted after the first weight-tile DMAs below.
            for dt in range(ND):
                nc.sync.dma_start(
                    out=xbf[dt][:, 0:DIM],
                    in_=xT.ap()[dt * P:(dt + 1) * P, 0:DIM])

            def load_x_peer_and_wv():
                for dt in range(ND):
                    nc.sync.dma_start(
                        out=xbf[dt][:, DIM:NJ],
                        in_=xT.ap()[dt * P:(dt + 1) * P, DIM:NJ])
                for dt in range(ND):
                    nc.sync.dma_start(
                        out=wv[dt],
                        in_=wqkvT.ap()[dt * P:(dt + 1) * P, 2 * DIM:3 * DIM])

            # ones columns of vaug (sum-exp trick)
            for jt in range(16):
                v3 = vaug[jt].rearrange("p (h c) -> p h c", c=AUG)
                nc.vector.memset(v3[:, :, HD:AUG], 1.0)

            def qk_proj(we, tiles, et, chunks):
                """Project one e-tile (2 heads) for the given seq chunks."""
                pss = {ch: pp.tile([P, DIM], FP32, tag="pp", name="pp")
                       for ch in chunks}
                for dt in range(ND):
                    for ch in chunks:
                        for sc in range(2):
                            nb = ch * DIM + sc * 512
                            nc.tensor.matmul(
                                pss[ch][:, sc * 512:(sc + 1) * 512],
                                we[:, dt, :],
                                xbf[dt][:, nb:nb + 512],
                                start=(dt == 0),
                                stop=(dt == ND - 1),
                            )
                for ch in chunks:
                    dst = tiles[et][:, ch * DIM:(ch + 1) * DIM]
                    if (et + ch) % 2 == 0:
                        nc.vector.tensor_copy(dst, pss[ch])
                    else:
                        nc.scalar.copy(dst, pss[ch])

            def v_proj(jt):
                ps = pp.tile([P, DIM], FP32, tag="pp", name="pp")
                for dt in range(ND):
                    for sc in range(2):  # e-chunks of 512 = 8 heads each
                        nc.tensor.matmul(
                            ps[:, sc * 512:(sc + 1) * 512],
                            xbf[dt][:, jt * P:(jt + 1) * P],
                            wv[dt][:, sc * 512:(sc + 1) * 512],
                            start=(dt == 0),
                            stop=(dt == ND - 1),
                        )
                vsrc = ps.rearrange("p (h c) -> p h c", c=HD)
                vdst = vaug[jt].rearrange("p (h c) -> p h c", c=AUG)[:, :, 0:HD]
                if jt % 2 == 0:
                    nc.vector.tensor_copy(vdst, vsrc)
                else:
                    nc.scalar.copy(vdst, vsrc)

            def emit_dots(pool, tag, hp, jt, half):
                """One head's dots [128 keys, 1024 queries] for key-tile jt.
                half 0 -> PE rows 0:64, half 1 -> rows 64:128; consecutive
                halves run concurrently on disjoint PE row ranges."""
                d = pool.tile([P, NI], FP32, tag=tag, name="dots")
                jsl = slice(jt * P, (jt + 1) * P)
                rsl = slice(0, HD) if half == 0 else slice(HD, P)
                for ic in range(2):
                    isl = slice(ic * 512, (ic + 1) * 512)
                    nc.tensor.matmul(
                        d[:, isl], kt[hp][rsl, jsl], qt[hp][rsl, isl],
                        start=True, stop=True,
                    )
                return d

            def emit_exp(d, e_t):
                nc.scalar.activation(e_t, d, mybir.ActivationFunctionType.Exp,
                                     scale=SCALE)

            # ---- projection phase, with pair-0 dots+exp (jt 0..NBUF-1)
            # interleaved; exps buffered in SBUF for AV replay later.
            wq0 = load_we(0, 0)
            wk0 = load_we(DIM, 0)
            load_x_peer_and_wv()
            qk_proj(wq0, qt, 0, [0])
            qk_proj(wk0, kt, 0, [0, 1])

            e0 = [e0pool.tile([P, NI], BF16, tag=f"e0_{j}", name=f"e0_{j}")
                  for j in range(2 * NBUF)]
            wjobs = [(0, e) for e in range(1, 8)] + \
                    [(DIM, e) for e in range(1, 8)]  # 14: Q et1-7, K et1-7
            wnext = load_we(*wjobs[0])
            for jt in range(16):
                if jt < NBUF:
                    dA = emit_dots(pp, 'pp', 0, jt, 0)
                    dB = emit_dots(pp, 'pp', 0, jt, 1)
                    emit_exp(dA, e0[2 * jt])
                    emit_exp(dB, e0[2 * jt + 1])
                v_proj(jt)
                if jt < 14:
                    wcur = wnext
                    if jt + 1 < 14:
                        wnext = load_we(*wjobs[jt + 1])
                    if jt < 7:
                        qk_proj(wcur, qt, jt + 1, [0])
                    else:
                        qk_proj(wcur, kt, jt - 6, [0, 1])

            pp_cm.__exit__(None, None, None)
            wpool_cm.__exit__(None, None, None)
            xpool_cm.__exit__(None, None, None)

            # ---- attention-phase pools (av pair 4 banks + dots 4 banks)
            late_cm = tc.tile_pool(name="late", bufs=1)
            late = late_cm.__enter__()
            aot = [late.tile([P, NI], BF16, tag=f"ao{e}", name=f"ao{e}")
                   for e in range(8)]
            wo = [late.tile([P, DIM], BF16, tag=f"wo{dt}", name=f"wo{dt}")
                  for dt in range(ND)]
            for dt in range(ND):
                nc.sync.dma_start(
                    out=wo[dt], in_=woutT.ap()[dt * P:(dt + 1) * P, :])

            norm_cm = tc.tile_pool(name="norm", bufs=2)
            norm = norm_cm.__enter__()
            pd_cm = tc.tile_pool(name="pd", bufs=2, space="PSUM")
            pd = pd_cm.__enter__()
            pav_cm = tc.tile_pool(name="pav", bufs=2, space="PSUM")
            pav = pav_cm.__enter__()

            def av_accum(av, e_t, jt, head):
                first, last = jt == 0, jt == 15
                for ic in range(2):
                    isl = slice(ic * 512, (ic + 1) * 512)
                    nc.tensor.matmul(
                        av[:, isl],
                        vaug[jt][:, head * AUG:(head + 1) * AUG],
                        e_t[:, isl],
                        start=first, stop=last,
                    )

            def heat(hp, n=6):
                # junk matmuls (no readers) bridge the pair-boundary PE gap:
                # without them the PE idles ~4us waiting for the av PSUM
                # release and HW DVFS drops it to the 1.2GHz mid p-state for
                # most of the next pair.
                pb = pd.tile([P, NI], FP32, tag="pd", name="heat")
                for i in range(n):
                    nc.tensor.matmul(
                        pb[:, (i % 2) * 512:(i % 2) * 512 + 512],
                        kt[hp][:, 0:P], qt[hp][:, 0:512],
                        start=True, stop=True,
                    )

            def normalize(hp, avA, avB):
                # per-pair softmax normalization (overlaps next pair).
                # Sum-exp rows gathered at partitions 0/32 (32-aligned as DVE
                # requires), one reciprocal per pair; partition_broadcast only
                # accepts base-0 inputs, so head B's row uses a base-0 temp.
                g = norm.tile([33, NI], FP32, tag="g", name="g")
                nc.vector.tensor_copy(aot[hp][0:HD, :], avA[0:HD, :])
                nc.vector.tensor_copy(aot[hp][HD:P, :], avB[0:HD, :])
                nc.vector.tensor_copy(g[0:1, :], avA[HD:AUG, :])
                nc.vector.tensor_copy(g[32:33, :], avB[HD:AUG, :])
                rp = norm.tile([33, NI], FP32, tag="rp", name="rp")
                nc.vector.reciprocal(rp, g)  # rows 1..31 junk, unused
                rbA = norm.tile([P, NI], FP32, tag="rb", name="rb")
                nc.gpsimd.partition_broadcast(rbA, rp[0:1, :])
                nc.gpsimd.tensor_mul(
                    aot[hp][0:HD, :], aot[hp][0:HD, :], rbA[0:HD, :])
                tb = norm.tile([1, NI], FP32, tag="tb", name="tb")
                nc.vector.tensor_copy(tb, rp[32:33, :])
                rbB = norm.tile([P, NI], FP32, tag="rb", name="rb")
                nc.gpsimd.partition_broadcast(rbB, tb)
                nc.gpsimd.tensor_mul(
                    aot[hp][HD:P, :], aot[hp][HD:P, :], rbB[HD:P, :])
                heat(hp)

            # ---- attention phase.
            # pair 0: AV replay of buffered jts + streamed jts NBUF..15.
            avA = pav.tile([AUG, NI], FP32, tag="pav", name="av")
            avB = pav.tile([AUG, NI], FP32, tag="pav", name="av")
            dA = emit_dots(pd, 'pd', 0, NBUF, 0)
            dB = emit_dots(pd, 'pd', 0, NBUF, 1)
            # replay first so AV start=True lands on jt 0 in PE order
            av_accum(avA, e0[0], 0, 0)
            av_accum(avB, e0[1], 0, 1)
            for jt in range(NBUF, 16):
                eA = sb.tile([P, NI], BF16, tag="expT", name="expT", bufs=4)
                eB = sb.tile([P, NI], BF16, tag="expT", name="expT", bufs=4)
                emit_exp(dA, eA)
                emit_exp(dB, eB)
                r = jt - NBUF + 1  # replay index
                if jt < 15:
                    dA = emit_dots(pd, 'pd', 0, jt + 1, 0)
                if r < NBUF:
                    av_accum(avA, e0[2 * r], r, 0)
                    av_accum(avB, e0[2 * r + 1], r, 1)
                av_accum(avA, eA, jt, 0)
                if jt < 15:
                    dB = emit_dots(pd, 'pd', 0, jt + 1, 1)
                av_accum(avB, eB, jt, 1)
            normalize(0, avA, avB)

            # pairs 1-7: lookahead-dots pipeline
            for hp in range(1, 8):
                avA = pav.tile([AUG, NI], FP32, tag="pav", name="av")
                avB = pav.tile([AUG, NI], FP32, tag="pav", name="av")
                dA = emit_dots(pd, 'pd', hp, 0, 0)
                dB = emit_dots(pd, 'pd', hp, 0, 1)
                for jt in range(16):
                    eA = sb.tile([P, NI], BF16, tag="expT", name="expT", bufs=4)
                    eB = sb.tile([P, NI], BF16, tag="expT", name="expT", bufs=4)
                    emit_exp(dA, eA)
                    emit_exp(dB, eB)
                    if jt < 15:
                        dA = emit_dots(pd, 'pd', hp, jt + 1, 0)
                    av_accum(avA, eA, jt, 2 * hp)
                    if jt < 15:
                        dB = emit_dots(pd, 'pd', hp, jt + 1, 1)
                    av_accum(avB, eB, jt, 2 * hp + 1)
                normalize(hp, avA, avB)

            pav_cm.__exit__(None, None, None)
            pd_cm.__exit__(None, None, None)
            norm_cm.__exit__(None, None, None)

            # ---- output projection + bias. Groups of 4 it-tiles; within a
            # group all et<7 matmuls go first so they overlap normalize(7)
            # (which produces aot[7]) instead of serializing behind it.
            po_cm = tc.tile_pool(name="po", bufs=4, space="PSUM")
            po = po_cm.__enter__()
            for g in range(2):
                its = range(4 * g, 4 * g + 4)
                pss = {}
                for it in its:
                    pss[it] = po.tile([P, DIM], FP32, tag="po", name="po")
                    for et in range(7):
                        for fc in range(2):
                            fsl = slice(fc * 512, (fc + 1) * 512)
                            nc.tensor.matmul(
                                pss[it][:, fsl],
                                aot[et][:, it * P:(it + 1) * P],
                                wo[et][:, fsl],
                                start=(et == 0),
                                stop=False,
                            )
                for it in its:
                    for fc in range(2):
                        fsl = slice(fc * 512, (fc + 1) * 512)
                        nc.tensor.matmul(
                            pss[it][:, fsl],
                            aot[7][:, it * P:(it + 1) * P],
                            wo[7][:, fsl],
                            start=False,
                            stop=True,
                        )
                    osb = sb.tile([P, DIM], FP32, tag="outsb", name="outsb",
                                  bufs=2)
                    nc.vector.tensor_add(osb, pss[it], bias_bc)
                    nc.sync.dma_start(
                        out=out.ap()[it * P:(it + 1) * P, :], in_=osb)
            po_cm.__exit__(None, None, None)
            late_cm.__exit__(None, None, None)
            e0pool_cm.__exit__(None, None, None)

    nc.compile()
    return nc


def _get_nc():
    global _NC_CACHE
    if _NC_CACHE is None:
        _NC_CACHE = _build()
    return _NC_CACHE


def kernel(x, w_qkv, w_out, b_out):
    global LAST_RESULTS
    import ml_dtypes
    BF = ml_dtypes.bfloat16
    x = np.asarray(x, dtype=np.float32)
    w_qkv = np.asarray(w_qkv, dtype=np.float32)
    w_out = np.asarray(w_out, dtype=np.float32)
    b_out = np.asarray(b_out, dtype=np.float32)

    nc = _get_nc()

    wqkvT = np.ascontiguousarray(w_qkv.T.astype(BF))
    woutT = np.ascontiguousarray(w_out.T.astype(BF))
    brow = np.ascontiguousarray(b_out.reshape(1, DIM))

    in_maps = []
    for c in range(N_CORES):
        b, h = divmod(c, 2)
        own = x[b, h * NI:(h + 1) * NI, :]
        peer = x[b, (1 - h) * NI:(2 - h) * NI, :]
        xTc = np.ascontiguousarray(
            np.concatenate([own, peer], axis=0).T.astype(BF))
        in_maps.append({
            "xT": xTc,
            "wqkvT": wqkvT,
            "woutT": woutT,
            "bout": brow,
        })

    res = run_bass_kernel_spmd(
        nc, in_maps, core_ids=list(range(N_CORES)), trace=TRACE
    )
    LAST_RESULTS = res

    out = np.empty((B, SEQ, DIM), dtype=np.float32)
    for c in range(N_CORES):
        b, h = divmod(c, 2)
        out[b, h * NI:(h + 1) * NI, :] = res.results[c]["out"]
    return out


# revision 15
# speedup vs baseline: 1.2609x; 1.0782x over previous
"""Multi-head attention (4x2048x1024, 16 heads) on 8 TRN2 NeuronCores.

Sharding: core c handles batch c//2, query seq-half c%2 (1024 queries).
Each core computes QKV projection for its own seq half plus K/V for the
peer half (redundant compute instead of a 2-rank collective), full
attention for all 16 heads over its 1024 queries x 2048 keys, and the
output projection. Outputs are disjoint -> no collectives; host concats.

v2 vs baseline:
- host pre-casts inputs to bf16: half the DMA bytes, no on-device
  stage+cast pipeline (frees DVE, removes weight-load stalls)
- heater bursts removed; ACT does exps only (proj copies on DVE/Pool)
- projection phase keeps PE saturated; pair-0 dots+exps interleaved
  into it with exp tiles buffered in SBUF (jt 0-7), so ACT starts early
- attention phase: per-pair lookahead-dots emission with dots PSUM
  double-buffered (pd 2x2 banks) + av pair (pav 2x2 banks) -> ACT
  (the bottleneck there) never waits on PSUM rotation
"""

import numpy as np

import concourse.mybir as mybir
import concourse.tile as tile
from concourse import bacc
from concourse.bass_utils import run_bass_kernel_spmd
FP32 = mybir.dt.float32
BF16 = mybir.dt.bfloat16

DIM = 1024
HEADS = 16
HD = 64
AUG = HD + 1  # V columns per head + ones column for sum-exp
SCALE = DIM ** -0.5
SEQ = 2048
NI = 1024  # queries per core
NJ = 2048  # keys per core
B = 4
N_CORES = 8
P = 128
ND = DIM // P  # 8 contraction tiles
NBUF = 8  # pair-0 jts with SBUF-buffered exps

TRACE = False
LAST_RESULTS = None
_NC_CACHE = None


def _build():
    nc = bacc.Bacc(
        "TRN2",
        target_bir_lowering=False,
        debug=False,
        enable_asserts=False,
        num_devices=N_CORES,
    )
    # all inputs pre-cast/transposed by host
    xT = nc.dram_tensor("xT", [DIM, NJ], BF16, kind="ExternalInput")
    wqkvT = nc.dram_tensor("wqkvT", [DIM, 3 * DIM], BF16, kind="ExternalInput")
    woutT = nc.dram_tensor("woutT", [DIM, DIM], BF16, kind="ExternalInput")
    bout = nc.dram_tensor("bout", [1, DIM], FP32, kind="ExternalInput")
    out = nc.dram_tensor("out", [NI, DIM], FP32, kind="ExternalOutput")

    with tile.TileContext(nc) as tc:
        with (
            tc.tile_pool(name="persist", bufs=1) as persist,
            tc.tile_pool(name="sb", bufs=3) as sb,
            tc.tile_pool(name="small", bufs=3) as small,
        ):
            e0pool_cm = tc.tile_pool(name="e0pool", bufs=1)
            e0pool = e0pool_cm.__enter__()
            xpool_cm = tc.tile_pool(name="xpool", bufs=1)
            xpool = xpool_cm.__enter__()
            wpool_cm = tc.tile_pool(name="wpool", bufs=1)
            wpool = wpool_cm.__enter__()
            pp_cm = tc.tile_pool(name="pp", bufs=4, space="PSUM")
            pp = pp_cm.__enter__()

            # ---- bias broadcast [1,1024] -> [128,1024]
            bias_sb = small.tile([1, DIM], FP32, tag="bias", name="bias", bufs=1)
            nc.sync.dma_start(out=bias_sb, in_=bout.ap())
            bias_bc = small.tile([P, DIM], FP32, tag="biasbc", name="biasbc", bufs=1)
            nc.gpsimd.partition_broadcast(bias_bc, bias_sb)

            # ---- persistent tiles
            xbf = [xpool.tile([P, NJ], BF16, tag=f"xbf{dt}", name=f"xbf{dt}")
                   for dt in range(ND)]
            qt = [persist.tile([P, NI], BF16, tag=f"qt{e}", name=f"qt{e}")
                  for e in range(8)]
            kt = [persist.tile([P, NJ], BF16, tag=f"kt{e}", name=f"kt{e}")
                  for e in range(8)]
            vaug = [persist.tile([P, HEADS * AUG], BF16, tag=f"va{j}",
                                 name=f"va{j}") for j in range(16)]

            # weights: wv full-width (v_proj uses all heads per jt); wq/wk
            # sliced per e-tile [128, (dt 8) x 128] (2KB/part) in rotating
            # pools, one strided DMA per e-tile, loaded just-in-time.
            wv = [wpool.tile([P, DIM], BF16, tag=f"wv{dt}", name=f"wv{dt}")
                  for dt in range(ND)]

            def load_we(ebase, et):
                """[128 p, 8 dt, 128 cols]: [p, dt, c] = wqkvT[dt*128+p,
                ebase + et*128 + c] -- one DMA for all 8 dt sub-tiles."""
                w = wpool.tile([P, ND, P], BF16, tag="we", name="we", bufs=5)
                nc.sync.dma_start(
                    out=w,
                    in_=wqkvT.ap()
                    .rearrange("(dt p) e -> p dt e", p=P)
                    [:, :, ebase + et * P: ebase + (et + 1) * P],
                )
                return w

            # DMA priority: first e-tiles of wq/wk, then x own half (these
            # gate the first projections), then x peer, then wv.

            def load_x_peer_and_wv():
                for dt in range(ND):
                    nc.sync.dma_start(
                        out=xbf[dt][:, DIM:NJ],
                        in_=xT.ap()[dt * P:(dt + 1) * P, DIM:NJ])
                for dt in range(ND):
                    nc.sync.dma_start(
                        out=wv[dt],
                        in_=wqkvT.ap()[dt * P:(dt + 1) * P, 2 * DIM:3 * DIM])

            wq0 = load_we(0, 0)
            wk0 = load_we(DIM, 0)
            for dt in range(ND):
                nc.sync.dma_start(
                    out=xbf[dt][:, 0:DIM],
                    in_=xT.ap()[dt * P:(dt + 1) * P, 0:DIM])
            load_x_peer_and_wv()

            # ones columns of vaug (sum-exp trick)
            for jt in range(16):
                v3 = vaug[jt].rearrange("p (h c) -> p h c", c=AUG)
                nc.vector.memset(v3[:, :, HD:AUG], 1.0)

            def qk_proj(we, tiles, et, chunks):
                """Project one e-tile (2 heads) for the given seq chunks."""
                pss = {ch: pp.tile([P, DIM], FP32, tag="pp", name="pp")
                       for ch in chunks}
                for dt in range(ND):
                    for ch in chunks:
                        for sc in range(2):
                            nb = ch * DIM + sc * 512
                            nc.tensor.matmul(
                                pss[ch][:, sc * 512:(sc + 1) * 512],
                                we[:, dt, :],
                                xbf[dt][:, nb:nb + 512],
                                start=(dt == 0),
                                stop=(dt == ND - 1),
                            )
                for ch in chunks:
                    dst = tiles[et][:, ch * DIM:(ch + 1) * DIM]
                    if (et + ch) % 2 == 0:
                        nc.vector.tensor_copy(dst, pss[ch])
                    else:
                        nc.scalar.copy(dst, pss[ch])

            def v_proj(jt):
                ps = pp.tile([P, DIM], FP32, tag="pp", name="pp")
                for dt in range(ND):
                    for sc in range(2):  # e-chunks of 512 = 8 heads each
                        nc.tensor.matmul(
                            ps[:, sc * 512:(sc + 1) * 512],
                            xbf[dt][:, jt * P:(jt + 1) * P],
                            wv[dt][:, sc * 512:(sc + 1) * 512],
                            start=(dt == 0),
                            stop=(dt == ND - 1),
                        )
                vsrc = ps.rearrange("p (h c) -> p h c", c=HD)
                vdst = vaug[jt].rearrange("p (h c) -> p h c", c=AUG)[:, :, 0:HD]
                if jt % 2 == 0:
                    nc.vector.tensor_copy(vdst, vsrc)
                else:
                    nc.scalar.copy(vdst, vsrc)

            def emit_dots(pool, tag, hp, jt, half):
                """One head's dots [128 keys, 1024 queries] for key-tile jt.
                half 0 -> PE rows 0:64, half 1 -> rows 64:128; consecutive
                halves run concurrently on disjoint PE row ranges."""
                d = pool.tile([P, NI], FP32, tag=tag, name="dots")
                jsl = slice(jt * P, (jt + 1) * P)
                rsl = slice(0, HD) if half == 0 else slice(HD, P)
                for ic in range(2):
                    isl = slice(ic * 512, (ic + 1) * 512)
                    nc.tensor.matmul(
                        d[:, isl], kt[hp][rsl, jsl], qt[hp][rsl, isl],
                        start=True, stop=True,
                    )
                return d

            def emit_exp(d, e_t):
                nc.scalar.activation(e_t, d, mybir.ActivationFunctionType.Exp,
                                     scale=SCALE)

            # ---- projection phase, with pair-0 dots+exp (jt 0..NBUF-1)
            # interleaved; exps buffered in SBUF for AV replay later.
            qk_proj(wq0, qt, 0, [0])
            qk_proj(wk0, kt, 0, [0, 1])

            e0 = [e0pool.tile([P, NI], BF16, tag=f"e0_{j}", name=f"e0_{j}")
                  for j in range(2 * NBUF)]
            wjobs = [(0, e) for e in range(1, 8)] + \
                    [(DIM, e) for e in range(1, 8)]  # 14: Q et1-7, K et1-7
            wnext = load_we(*wjobs[0])
            for jt in range(16):
                if jt < NBUF:
                    dA = emit_dots(pp, 'pp', 0, jt, 0)
                    dB = emit_dots(pp, 'pp', 0, jt, 1)
                    emit_exp(dA, e0[2 * jt])
                    emit_exp(dB, e0[2 * jt + 1])
                v_proj(jt)
                if jt < 14:
                    wcur = wnext
                    if jt + 1 < 14:
                        wnext = load_we(*wjobs[jt + 1])
                    if jt < 7:
                        qk_proj(wcur, qt, jt + 1, [0])
                    else:
                        qk_proj(wcur, kt, jt - 6, [0, 1])

            pp_cm.__exit__(None, None, None)
            wpool_cm.__exit__(None, None, None)
            xpool_cm.__exit__(None, None, None)

            # ---- attention-phase pools (av pair 4 banks + dots 4 banks)
            late_cm = tc.tile_pool(name="late", bufs=1)
            late = late_cm.__enter__()
            aot = [late.tile([P, NI], BF16, tag=f"ao{e}", name=f"ao{e}")
                   for e in range(8)]
            wo = [late.tile([P, DIM], BF16, tag=f"wo{dt}", name=f"wo{dt}")
                  for dt in range(ND)]
            for dt in range(ND):
                nc.sync.dma_start(
                    out=wo[dt], in_=woutT.ap()[dt * P:(dt + 1) * P, :])

            norm_cm = tc.tile_pool(name="norm", bufs=2)
            norm = norm_cm.__enter__()
            pd_cm = tc.tile_pool(name="pd", bufs=2, space="PSUM")
            pd = pd_cm.__enter__()
            pav_cm = tc.tile_pool(name="pav", bufs=2, space="PSUM")
            pav = pav_cm.__enter__()

            def av_accum(av, e_t, jt, head):
                first, last = jt == 0, jt == 15
                for ic in range(2):
                    isl = slice(ic * 512, (ic + 1) * 512)
                    nc.tensor.matmul(
                        av[:, isl],
                        vaug[jt][:, head * AUG:(head + 1) * AUG],
                        e_t[:, isl],
                        start=first, stop=last,
                    )

            def heat(hp, n=10):
                # junk matmuls (no readers) bridge the pair-boundary PE gap:
                # without them the PE idles ~4us waiting for the av PSUM
                # release and HW DVFS drops it to the 1.2GHz mid p-state for
                # most of the next pair.
                pb = pd.tile([P, NI], FP32, tag="pd", name="heat")
                for i in range(n):
                    nc.tensor.matmul(
                        pb[:, (i % 2) * 512:(i % 2) * 512 + 512],
                        kt[hp][:, 0:P], qt[hp][:, 0:512],
                        start=True, stop=True,
                    )

            def normalize(hp, avA, avB):
                # per-pair softmax normalization (overlaps next pair).
                # Sum-exp rows gathered at partitions 0/32 (32-aligned as DVE
                # requires); one bf16 reciprocal per pair; the broadcast along
                # partitions runs on the PE (ones-stationary matmul into a pd
                # tile) and the scaling muls on DVE -- GpSimd's slow semaphore
                # path added ~15us per pair when these lived there.
                g = norm.tile([33, NI], FP32, tag="g", name="g")
                nc.vector.tensor_copy(aot[hp][0:HD, :], avA[0:HD, :])
                nc.vector.tensor_copy(g[0:1, :], avA[HD:AUG, :])
                nc.vector.tensor_copy(aot[hp][HD:P, :], avB[0:HD, :])
                nc.vector.tensor_copy(g[32:33, :], avB[HD:AUG, :])
                rp = norm.tile([33, NI], FP32, tag="rp", name="rp")
                nc.vector.reciprocal(rp, g)  # rows 1..31 junk, unused
                rbA = norm.tile([P, NI], FP32, tag="rb", name="rb")
                nc.gpsimd.partition_broadcast(rbA, rp[0:1, :])
                nc.vector.tensor_mul(
                    aot[hp][0:HD, :], aot[hp][0:HD, :], rbA[0:HD, :])
                tb = norm.tile([1, NI], FP32, tag="tb", name="tb")
                nc.vector.tensor_copy(tb, rp[32:33, :])
                rbB = norm.tile([P, NI], FP32, tag="rb", name="rb")
                nc.gpsimd.partition_broadcast(rbB, tb)
                nc.vector.tensor_mul(
                    aot[hp][HD:P, :], aot[hp][HD:P, :], rbB[HD:P, :])
                if hp < 7:
                    heat(hp)

            # ---- attention phase.
            # pair 0: AV replay of buffered jts + streamed jts NBUF..15.
            avA = pav.tile([AUG, NI], FP32, tag="pav", name="av")
            avB = pav.tile([AUG, NI], FP32, tag="pav", name="av")
            dA = emit_dots(pd, 'pd', 0, NBUF, 0)
            dB = emit_dots(pd, 'pd', 0, NBUF, 1)
            # replay first so AV start=True lands on jt 0 in PE order
            av_accum(avA, e0[0], 0, 0)
            av_accum(avB, e0[1], 0, 1)
            for jt in range(NBUF, 16):
                eA = sb.tile([P, NI], BF16, tag="expT", name="expT", bufs=4)
                eB = sb.tile([P, NI], BF16, tag="expT", name="expT", bufs=4)
                emit_exp(dA, eA)
                emit_exp(dB, eB)
                r = jt - NBUF + 1  # replay index
                if jt < 15:
                    dA = emit_dots(pd, 'pd', 0, jt + 1, 0)
                if r < NBUF:
                    av_accum(avA, e0[2 * r], r, 0)
                    av_accum(avB, e0[2 * r + 1], r, 1)
                av_accum(avA, eA, jt, 0)
                if jt < 15:
                    dB = emit_dots(pd, 'pd', 0, jt + 1, 1)
                av_accum(avB, eB, jt, 1)
            normalize(0, avA, avB)

            # pairs 1-7: lookahead-dots pipeline
            for hp in range(1, 8):
                avA = pav.tile([AUG, NI], FP32, tag="pav", name="av")
                avB = pav.tile([AUG, NI], FP32, tag="pav", name="av")
                dA = emit_dots(pd, 'pd', hp, 0, 0)
                dB = emit_dots(pd, 'pd', hp, 0, 1)
                for jt in range(16):
                    eA = sb.tile([P, NI], BF16, tag="expT", name="expT", bufs=4)
                    eB = sb.tile([P, NI], BF16, tag="expT", name="expT", bufs=4)
                    emit_exp(dA, eA)
                    emit_exp(dB, eB)
                    if jt < 15:
                        dA = emit_dots(pd, 'pd', hp, jt + 1, 0)
                    av_accum(avA, eA, jt, 2 * hp)
                    if jt < 15:
                        dB = emit_dots(pd, 'pd', hp, jt + 1, 1)
                    av_accum(avB, eB, jt, 2 * hp + 1)
                normalize(hp, avA, avB)

            pav_cm.__exit__(None, None, None)
            pd_cm.__exit__(None, None, None)
            norm_cm.__exit__(None, None, None)

            # ---- output projection + bias. Groups of 4 it-tiles; within a
            # group all et<7 matmuls go first so they overlap normalize(7)
            # (which produces aot[7]) instead of serializing behind it.
            po_cm = tc.tile_pool(name="po", bufs=4, space="PSUM")
            po = po_cm.__enter__()
            for g in range(2):
                its = range(4 * g, 4 * g + 4)
                pss = {}
                for it in its:
                    pss[it] = po.tile([P, DIM], FP32, tag="po", name="po")
                    for et in range(7):
                        for fc in range(2):
                            fsl = slice(fc * 512, (fc + 1) * 512)
                            nc.tensor.matmul(
                                pss[it][:, fsl],
                                aot[et][:, it * P:(it + 1) * P],
                                wo[et][:, fsl],
                                start=(et == 0),
                                stop=False,
                            )
                for it in its:
                    for fc in range(2):
                        fsl = slice(fc * 512, (fc + 1) * 512)
                        nc.tensor.matmul(
                            pss[it][:, fsl],
                            aot[7][:, it * P:(it + 1) * P],
                            wo[7][:, fsl],
                            start=False,
                            stop=True,
                        )
                    osb = sb.tile([P, DIM], FP32, tag="outsb", name="outsb",
                                  bufs=2)
                    nc.vector.tensor_add(osb, pss[it], bias_bc)
                    nc.sync.dma_start(
                        out=out.ap()[it * P:(it + 1) * P, :], in_=osb)
            po_cm.__exit__(None, None, None)
            late_cm.__exit__(None, None, None)
            e0pool_cm.__exit__(None, None, None)

    nc.compile()
    return nc


def _get_nc():
    global _NC_CACHE
    if _NC_CACHE is None:
        _NC_CACHE = _build()
    return _NC_CACHE


def kernel(x, w_qkv, w_out, b_out):
    global LAST_RESULTS
    import ml_dtypes
    BF = ml_dtypes.bfloat16
    x = np.asarray(x, dtype=np.float32)
    w_qkv = np.asarray(w_qkv, dtype=np.float32)
    w_out = np.asarray(w_out, dtype=np.float32)
    b_out = np.asarray(b_out, dtype=np.float32)

    nc = _get_nc()

    wqkvT = np.ascontiguousarray(w_qkv.T.astype(BF))
    woutT = np.ascontiguousarray(w_out.T.astype(BF))
    brow = np.ascontiguousarray(b_out.reshape(1, DIM))

    in_maps = []
    for c in range(N_CORES):
        b, h = divmod(c, 2)
        own = x[b, h * NI:(h + 1) * NI, :]
        peer = x[b, (1 - h) * NI:(2 - h) * NI, :]
        xTc = np.ascontiguousarray(
            np.concatenate([own, peer], axis=0).T.astype(BF))
        in_maps.append({
            "xT": xTc,
            "wqkvT": wqkvT,
            "woutT": woutT,
            "bout": brow,
        })

    res = run_bass_kernel_spmd(
        nc, in_maps, core_ids=list(range(N_CORES)), trace=TRACE
    )
    LAST_RESULTS = res

    out = np.empty((B, SEQ, DIM), dtype=np.float32)
    for c in range(N_CORES):
        b, h = divmod(c, 2)
        out[b, h * NI:(h + 1) * NI, :] = res.results[c]["out"]
    return out


# revision 16
# speedup vs baseline: 1.2673x; 1.0050x over previous
"""Multi-head attention (4x2048x1024, 16 heads) on 8 TRN2 NeuronCores.

Sharding: core c handles batch c//2, query seq-half c%2 (1024 queries).
Each core computes QKV projection for its own seq half plus K/V for the
peer half (redundant compute instead of a 2-rank collective), full
attention for all 16 heads over its 1024 queries x 2048 keys, and the
output projection. Outputs are disjoint -> no collectives; host concats.

v2 vs baseline:
- host pre-casts inputs to bf16: half the DMA bytes, no on-device
  stage+cast pipeline (frees DVE, removes weight-load stalls)
- heater bursts removed; ACT does exps only (proj copies on DVE/Pool)
- projection phase keeps PE saturated; pair-0 dots+exps interleaved
  into it with exp tiles buffered in SBUF (jt 0-7), so ACT starts early
- attention phase: per-pair lookahead-dots emission with dots PSUM
  double-buffered (pd 2x2 banks) + av pair (pav 2x2 banks) -> ACT
  (the bottleneck there) never waits on PSUM rotation
"""

import numpy as np

import concourse.mybir as mybir
import concourse.tile as tile
from concourse import bacc
from concourse.bass_utils import run_bass_kernel_spmd
FP32 = mybir.dt.float32
BF16 = mybir.dt.bfloat16

DIM = 1024
HEADS = 16
HD = 64
AUG = HD + 1  # V columns per head + ones column for sum-exp
SCALE = DIM ** -0.5
SEQ = 2048
NI = 1024  # queries per core
NJ = 2048  # keys per core
B = 4
N_CORES = 8
P = 128
ND = DIM // P  # 8 contraction tiles
NBUF = 8  # pair-0 jts with SBUF-buffered exps

TRACE = False
LAST_RESULTS = None
_NC_CACHE = None


def _build():
    nc = bacc.Bacc(
        "TRN2",
        target_bir_lowering=False,
        debug=False,
        enable_asserts=False,
        num_devices=N_CORES,
    )
    # all inputs pre-cast/transposed by host
    xT = nc.dram_tensor("xT", [DIM, NJ], BF16, kind="ExternalInput")
    wqkvT = nc.dram_tensor("wqkvT", [DIM, 3 * DIM], BF16, kind="ExternalInput")
    woutT = nc.dram_tensor("woutT", [DIM, DIM], BF16, kind="ExternalInput")
    bout = nc.dram_tensor("bout", [1, DIM], FP32, kind="ExternalInput")
    out = nc.dram_tensor("out", [NI, DIM], FP32, kind="ExternalOutput")

    with tile.TileContext(nc) as tc:
        with (
            tc.tile_pool(name="persist", bufs=1) as persist,
            tc.tile_pool(name="sb", bufs=3) as sb,
            tc.tile_pool(name="small", bufs=3) as small,
        ):
            e0pool_cm = tc.tile_pool(name="e0pool", bufs=1)
            e0pool = e0pool_cm.__enter__()
            xpool_cm = tc.tile_pool(name="xpool", bufs=1)
            xpool = xpool_cm.__enter__()
            wpool_cm = tc.tile_pool(name="wpool", bufs=1)
            wpool = wpool_cm.__enter__()
            pp_cm = tc.tile_pool(name="pp", bufs=4, space="PSUM")
            pp = pp_cm.__enter__()

            # ---- bias broadcast [1,1024] -> [128,1024]
            bias_sb = small.tile([1, DIM], FP32, tag="bias", name="bias", bufs=1)
            nc.sync.dma_start(out=bias_sb, in_=bout.ap())
            bias_bc = small.tile([P, DIM], FP32, tag="biasbc", name="biasbc", bufs=1)
            nc.gpsimd.partition_broadcast(bias_bc, bias_sb)

            # ---- persistent tiles
            xbf = [xpool.tile([P, NJ], BF16, tag=f"xbf{dt}", name=f"xbf{dt}")
                   for dt in range(ND)]
            qt = [persist.tile([P, NI], BF16, tag=f"qt{e}", name=f"qt{e}")
                  for e in range(8)]
            kt = [persist.tile([P, NJ], BF16, tag=f"kt{e}", name=f"kt{e}")
                  for e in range(8)]
            vaug = [persist.tile([P, HEADS * AUG], BF16, tag=f"va{j}",
                                 name=f"va{j}") for j in range(16)]

            # weights: wv full-width (v_proj uses all heads per jt); wq/wk
            # sliced per e-tile [128, (dt 8) x 128] (2KB/part) in rotating
            # pools, one strided DMA per e-tile, loaded just-in-time.
            wv = [wpool.tile([P, DIM], BF16, tag=f"wv{dt}", name=f"wv{dt}")
                  for dt in range(ND)]

            def load_we(ebase, et):
                """[128 p, 8 dt, 128 cols]: [p, dt, c] = wqkvT[dt*128+p,
                ebase + et*128 + c] -- one DMA for all 8 dt sub-tiles."""
                w = wpool.tile([P, ND, P], BF16, tag="we", name="we", bufs=5)
                nc.sync.dma_start(
                    out=w,
                    in_=wqkvT.ap()
                    .rearrange("(dt p) e -> p dt e", p=P)
                    [:, :, ebase + et * P: ebase + (et + 1) * P],
                )
                return w

            # DMA priority: first e-tiles of wq/wk, then x own half (these
            # gate the first projections), then x peer, then wv.

            def load_x_peer_and_wv():
                for dt in range(ND):
                    nc.sync.dma_start(
                        out=xbf[dt][:, DIM:NJ],
                        in_=xT.ap()[dt * P:(dt + 1) * P, DIM:NJ])
                for dt in range(ND):
                    nc.sync.dma_start(
                        out=wv[dt],
                        in_=wqkvT.ap()[dt * P:(dt + 1) * P, 2 * DIM:3 * DIM])

            wq0 = load_we(0, 0)
            wk0 = load_we(DIM, 0)
            for dt in range(ND):
                nc.sync.dma_start(
                    out=xbf[dt][:, 0:DIM],
                    in_=xT.ap()[dt * P:(dt + 1) * P, 0:DIM])
            load_x_peer_and_wv()

            # ones columns of vaug (sum-exp trick)
            for jt in range(16):
                v3 = vaug[jt].rearrange("p (h c) -> p h c", c=AUG)
                nc.vector.memset(v3[:, :, HD:AUG], 1.0)

            def qk_proj(we, tiles, et, chunks):
                """Project one e-tile (2 heads) for the given seq chunks."""
                pss = {ch: pp.tile([P, DIM], FP32, tag="pp", name="pp")
                       for ch in chunks}
                for dt in range(ND):
                    for ch in chunks:
                        for sc in range(2):
                            nb = ch * DIM + sc * 512
                            nc.tensor.matmul(
                                pss[ch][:, sc * 512:(sc + 1) * 512],
                                we[:, dt, :],
                                xbf[dt][:, nb:nb + 512],
                                start=(dt == 0),
                                stop=(dt == ND - 1),
                            )
                for ch in chunks:
                    dst = tiles[et][:, ch * DIM:(ch + 1) * DIM]
                    if (et + ch) % 2 == 0:
                        nc.vector.tensor_copy(dst, pss[ch])
                    else:
                        nc.scalar.copy(dst, pss[ch])

            def v_proj(jt):
                ps = pp.tile([P, DIM], FP32, tag="pp", name="pp")
                for dt in range(ND):
                    for sc in range(2):  # e-chunks of 512 = 8 heads each
                        nc.tensor.matmul(
                            ps[:, sc * 512:(sc + 1) * 512],
                            xbf[dt][:, jt * P:(jt + 1) * P],
                            wv[dt][:, sc * 512:(sc + 1) * 512],
                            start=(dt == 0),
                            stop=(dt == ND - 1),
                        )
                vsrc = ps.rearrange("p (h c) -> p h c", c=HD)
                vdst = vaug[jt].rearrange("p (h c) -> p h c", c=AUG)[:, :, 0:HD]
                if jt % 2 == 0:
                    nc.vector.tensor_copy(vdst, vsrc)
                else:
                    nc.scalar.copy(vdst, vsrc)

            def emit_dots(pool, tag, hp, jt, half):
                """One head's dots [128 keys, 1024 queries] for key-tile jt.
                half 0 -> PE rows 0:64, half 1 -> rows 64:128; consecutive
                halves run concurrently on disjoint PE row ranges."""
                d = pool.tile([P, NI], FP32, tag=tag, name="dots")
                jsl = slice(jt * P, (jt + 1) * P)
                rsl = slice(0, HD) if half == 0 else slice(HD, P)
                for ic in range(2):
                    isl = slice(ic * 512, (ic + 1) * 512)
                    nc.tensor.matmul(
                        d[:, isl], kt[hp][rsl, jsl], qt[hp][rsl, isl],
                        start=True, stop=True,
                    )
                return d

            def emit_exp(d, e_t):
                nc.scalar.activation(e_t, d, mybir.ActivationFunctionType.Exp,
                                     scale=SCALE)

            # ---- projection phase, with pair-0 dots+exp (jt 0..NBUF-1)
            # interleaved; exps buffered in SBUF for AV replay later.
            qk_proj(wq0, qt, 0, [0])
            qk_proj(wk0, kt, 0, [0])
            qk_proj(wk0, kt, 0, [1])

            e0 = [e0pool.tile([P, NI], BF16, tag=f"e0_{j}", name=f"e0_{j}")
                  for j in range(2 * NBUF)]
            wjobs = [(0, e) for e in range(1, 8)] + \
                    [(DIM, e) for e in range(1, 8)]  # 14: Q et1-7, K et1-7
            wnext = load_we(*wjobs[0])
            for jt in range(16):
                if jt < NBUF:
                    dA = emit_dots(pp, 'pp', 0, jt, 0)
                    dB = emit_dots(pp, 'pp', 0, jt, 1)
                    emit_exp(dA, e0[2 * jt])
                    emit_exp(dB, e0[2 * jt + 1])
                v_proj(jt)
                if jt < 14:
                    wcur = wnext
                    if jt + 1 < 14:
                        wnext = load_we(*wjobs[jt + 1])
                    if jt < 7:
                        qk_proj(wcur, qt, jt + 1, [0])
                    else:
                        qk_proj(wcur, kt, jt - 6, [0, 1])

            pp_cm.__exit__(None, None, None)
            wpool_cm.__exit__(None, None, None)
            xpool_cm.__exit__(None, None, None)

            # ---- attention-phase pools (av pair 4 banks + dots 4 banks)
            late_cm = tc.tile_pool(name="late", bufs=1)
            late = late_cm.__enter__()
            aot = [late.tile([P, NI], BF16, tag=f"ao{e}", name=f"ao{e}")
                   for e in range(8)]
            wo = [late.tile([P, DIM], BF16, tag=f"wo{dt}", name=f"wo{dt}")
                  for dt in range(ND)]
            for dt in range(ND):
                nc.sync.dma_start(
                    out=wo[dt], in_=woutT.ap()[dt * P:(dt + 1) * P, :])

            norm_cm = tc.tile_pool(name="norm", bufs=2)
            norm = norm_cm.__enter__()
            pd_cm = tc.tile_pool(name="pd", bufs=2, space="PSUM")
            pd = pd_cm.__enter__()
            pav_cm = tc.tile_pool(name="pav", bufs=2, space="PSUM")
            pav = pav_cm.__enter__()

            def av_accum(av, e_t, jt, head):
                first, last = jt == 0, jt == 15
                for ic in range(2):
                    isl = slice(ic * 512, (ic + 1) * 512)
                    nc.tensor.matmul(
                        av[:, isl],
                        vaug[jt][:, head * AUG:(head + 1) * AUG],
                        e_t[:, isl],
                        start=first, stop=last,
                    )

            def heat(hp, n=10):
                # junk matmuls (no readers) bridge the pair-boundary PE gap:
                # without them the PE idles ~4us waiting for the av PSUM
                # release and HW DVFS drops it to the 1.2GHz mid p-state for
                # most of the next pair.
                pb = pd.tile([P, NI], FP32, tag="pd", name="heat")
                for i in range(n):
                    nc.tensor.matmul(
                        pb[:, (i % 2) * 512:(i % 2) * 512 + 512],
                        kt[hp][:, 0:P], qt[hp][:, 0:512],
                        start=True, stop=True,
                    )

            def normalize(hp, avA, avB):
                # per-pair softmax normalization (overlaps next pair).
                # Sum-exp rows gathered at partitions 0/32 (32-aligned as DVE
                # requires); one bf16 reciprocal per pair; the broadcast along
                # partitions runs on the PE (ones-stationary matmul into a pd
                # tile) and the scaling muls on DVE -- GpSimd's slow semaphore
                # path added ~15us per pair when these lived there.
                g = norm.tile([33, NI], FP32, tag="g", name="g")
                nc.vector.tensor_copy(g[0:1, :], avA[HD:AUG, :])
                nc.vector.tensor_copy(g[32:33, :], avB[HD:AUG, :])
                nc.vector.tensor_copy(aot[hp][0:HD, :], avA[0:HD, :])
                nc.vector.tensor_copy(aot[hp][HD:P, :], avB[0:HD, :])
                rp = norm.tile([33, NI], FP32, tag="rp", name="rp")
                nc.vector.reciprocal(rp, g)  # rows 1..31 junk, unused
                rbA = norm.tile([P, NI], FP32, tag="rb", name="rb")
                nc.gpsimd.partition_broadcast(rbA, rp[0:1, :])
                nc.vector.tensor_mul(
                    aot[hp][0:HD, :], aot[hp][0:HD, :], rbA[0:HD, :])
                tb = norm.tile([1, NI], FP32, tag="tb", name="tb")
                nc.vector.tensor_copy(tb, rp[32:33, :])
                rbB = norm.tile([P, NI], FP32, tag="rb", name="rb")
                nc.gpsimd.partition_broadcast(rbB, tb)
                nc.vector.tensor_mul(
                    aot[hp][HD:P, :], aot[hp][HD:P, :], rbB[HD:P, :])

            # ---- attention phase.
            # pair 0: AV replay of buffered jts + streamed jts NBUF..15.
            avA = pav.tile([AUG, NI], FP32, tag="pav", name="av")
            avB = pav.tile([AUG, NI], FP32, tag="pav", name="av")
            dA = emit_dots(pd, 'pd', 0, NBUF, 0)
            dB = emit_dots(pd, 'pd', 0, NBUF, 1)
            # replay first so AV start=True lands on jt 0 in PE order
            av_accum(avA, e0[0], 0, 0)
            av_accum(avB, e0[1], 0, 1)
            for jt in range(NBUF, 16):
                eA = sb.tile([P, NI], BF16, tag="expT", name="expT", bufs=4)
                eB = sb.tile([P, NI], BF16, tag="expT", name="expT", bufs=4)
                emit_exp(dA, eA)
                emit_exp(dB, eB)
                r = jt - NBUF + 1  # replay index
                if jt < 15:
                    dA = emit_dots(pd, 'pd', 0, jt + 1, 0)
                if r < NBUF:
                    av_accum(avA, e0[2 * r], r, 0)
                    av_accum(avB, e0[2 * r + 1], r, 1)
                av_accum(avA, eA, jt, 0)
                if jt < 15:
                    dB = emit_dots(pd, 'pd', 0, jt + 1, 1)
                av_accum(avB, eB, jt, 1)
            normalize(0, avA, avB)

            # pairs 1-7: lookahead-dots pipeline
            for hp in range(1, 8):
                avA = pav.tile([AUG, NI], FP32, tag="pav", name="av")
                avB = pav.tile([AUG, NI], FP32, tag="pav", name="av")
                dA = emit_dots(pd, 'pd', hp, 0, 0)
                dB = emit_dots(pd, 'pd', hp, 0, 1)
                heat(hp)
                for jt in range(16):
                    eA = sb.tile([P, NI], BF16, tag="expT", name="expT", bufs=4)
                    eB = sb.tile([P, NI], BF16, tag="expT", name="expT", bufs=4)
                    emit_exp(dA, eA)
                    emit_exp(dB, eB)
                    if jt < 15:
                        dA = emit_dots(pd, 'pd', hp, jt + 1, 0)
                    av_accum(avA, eA, jt, 2 * hp)
                    if jt < 15:
                        dB = emit_dots(pd, 'pd', hp, jt + 1, 1)
                    av_accum(avB, eB, jt, 2 * hp + 1)
                normalize(hp, avA, avB)

            pav_cm.__exit__(None, None, None)
            pd_cm.__exit__(None, None, None)
            norm_cm.__exit__(None, None, None)

            # ---- output projection + bias. Groups of 4 it-tiles; within a
            # group all et<7 matmuls go first so they overlap normalize(7)
            # (which produces aot[7]) instead of serializing behind it.
            po_cm = tc.tile_pool(name="po", bufs=4, space="PSUM")
            po = po_cm.__enter__()
            for g in range(2):
                its = range(4 * g, 4 * g + 4)
                pss = {}
                for it in its:
                    pss[it] = po.tile([P, DIM], FP32, tag="po", name="po")
                    for et in range(7):
                        for fc in range(2):
                            fsl = slice(fc * 512, (fc + 1) * 512)
                            nc.tensor.matmul(
                                pss[it][:, fsl],
                                aot[et][:, it * P:(it + 1) * P],
                                wo[et][:, fsl],
                                start=(et == 0),
                                stop=False,
                            )
                for it in its:
                    for fc in range(2):
                        fsl = slice(fc * 512, (fc + 1) * 512)
                        nc.tensor.matmul(
                            pss[it][:, fsl],
                            aot[7][:, it * P:(it + 1) * P],
                            wo[7][:, fsl],
                            start=False,
                            stop=True,
                        )
                    osb = sb.tile([P, DIM], FP32, tag="outsb", name="outsb",
                                  bufs=2)
                    nc.vector.tensor_add(osb, pss[it], bias_bc)
                    nc.sync.dma_start(
                        out=out.ap()[it * P:(it + 1) * P, :], in_=osb)
            po_cm.__exit__(None, None, None)
            late_cm.__exit__(None, None, None)
            e0pool_cm.__exit__(None, None, None)

    nc.compile()
    return nc


def _get_nc():
    global _NC_CACHE
    if _NC_CACHE is None:
        _NC_CACHE = _build()
    return _NC_CACHE


def kernel(x, w_qkv, w_out, b_out):
    global LAST_RESULTS
    import ml_dtypes
    BF = ml_dtypes.bfloat16
    x = np.asarray(x, dtype=np.float32)
    w_qkv = np.asarray(w_qkv, dtype=np.float32)
    w_out = np.asarray(w_out, dtype=np.float32)
    b_out = np.asarray(b_out, dtype=np.float32)

    nc = _get_nc()

    wqkvT = np.ascontiguousarray(w_qkv.T.astype(BF))
    woutT = np.ascontiguousarray(w_out.T.astype(BF))
    brow = np.ascontiguousarray(b_out.reshape(1, DIM))

    in_maps = []
    for c in range(N_CORES):
        b, h = divmod(c, 2)
        own = x[b, h * NI:(h + 1) * NI, :]
        peer = x[b, (1 - h) * NI:(2 - h) * NI, :]
        xTc = np.ascontiguousarray(
            np.concatenate([own, peer], axis=0).T.astype(BF))
        in_maps.append({
            "xT": xTc,
            "wqkvT": wqkvT,
            "woutT": woutT,
            "bout": brow,
        })

    res = run_bass_kernel_spmd(
        nc, in_maps, core_ids=list(range(N_CORES)), trace=TRACE
    )
    LAST_RESULTS = res

    out = np.empty((B, SEQ, DIM), dtype=np.float32)
    for c in range(N_CORES):
        b, h = divmod(c, 2)
        out[b, h * NI:(h + 1) * NI, :] = res.results[c]["out"]
    return out
